# revision 13
# baseline (speedup 1.0000x reference)
"""Trainium2 Bass kernel for nn_AvgPoolVectorsPerWSI (segment-mean over groups).

Math: x [N=2048, M=512, 7, 7], idx [N] in [0,64)
  out[g, m] = mean over {n: idx[n]==g} and spatial of x[n, m, :, :]  -> [64, 512, 1, 1]

Strategy (no collectives needed):
  - Shard over M: core k handles an m-slice of 64 channels. Each core reads
    its x slice [2048, 64, 49] (25.7 MB) once -> memory-bound (~71 us/core).
  - Per 128-row n-tile: VectorE reduces spatial j (fp32 exact, overlapped
    under the DMA stream), then one small fp32 matmul with the one-hot
    segment matrix accumulates group sums into PSUM:
      psum[g, m] += w[n, g]^T @ xs[n, m]
  - Epilogue: multiply by per-group scale 1/(count_g*49) (host-computed),
    DMA out [64, 64]. Host concatenates the 8 results along m.

Raw Block implementation (not Tile): the walrus matmul/DMA lowerings only
accept ONE attached sync-wait per instruction; standalone wait_ge
instructions sidestep that. One semaphore per x-tile DMA: a cumulative
count over a shared sem can be satisfied by LATER tiles' completions while
tile t still has a lagging SDMA engine (8-partition stripe) in flight.
"""

from contextlib import ExitStack

import numpy as np

import concourse.bass as bass
import concourse.mybir as mybir
from concourse.bass_utils import run_bass_kernel_spmd

N = 2048          # samples
M = 512           # channels
HW = 49           # spatial (7*7)
G = 64            # groups
CORES = 8
ML = M // CORES   # 64 channels per core
F = ML * HW       # 3136 floats per (n, core)
P = 128           # partitions per tile
NT = N // P       # 16 n-tiles
BUFS = 6          # x-tile double-buffer depth
XBUFS = 4         # reduced-tile (xs) buffer depth

F32 = mybir.dt.float32


def _build():
    nc = bass.Bass(trn_type="TRN2", target_bir_lowering=False)
    x_ext = nc.declare_dram_parameter("x", [N, F], F32, isOutput=False)
    w_ext = nc.declare_dram_parameter("w", [P, NT * G], F32, isOutput=False)
    s_ext = nc.declare_dram_parameter("s", [G, 1], F32, isOutput=False)
    out_ext = nc.declare_dram_parameter("out", [G, ML], F32, isOutput=True)

    x_t = x_ext.ap().rearrange("(t p) f -> t p f", p=P)  # [16, 128, 3136]

    with ExitStack() as ctx:
        x_buf = ctx.enter_context(nc.sbuf_tensor([P, BUFS * F], F32))
        xs_buf = ctx.enter_context(nc.sbuf_tensor([P, XBUFS * ML], F32))
        w_sb = ctx.enter_context(nc.sbuf_tensor([P, NT * G], F32))
        s_sb = ctx.enter_context(nc.sbuf_tensor([G, 1], F32))
        out_sb = ctx.enter_context(nc.sbuf_tensor([G, ML], F32))
        psum = ctx.enter_context(nc.psum_tensor([G, ML], F32))
        dma_x = [
            ctx.enter_context(nc.semaphore(name=f"dma_x{t}")) for t in range(NT)
        ]
        dma_w = ctx.enter_context(nc.semaphore())   # +16 when w resident
        dma_s = ctx.enter_context(nc.semaphore())   # +16 when s resident
        dma_o = ctx.enter_context(nc.semaphore())   # +16 when out written
        red_sem = ctx.enter_context(nc.semaphore())  # +1 per reduced n-tile
        pe_sem = ctx.enter_context(nc.semaphore())   # +1 per accumulated n-tile
        fin_sem = ctx.enter_context(nc.semaphore())  # +1 when out_sb ready
        block = ctx.enter_context(nc.Block())

        @block.sync
        def _(sync):
            sync.dma_start(out=w_sb[:, :], in_=w_ext.ap()).then_inc(dma_w, 16)
            sync.dma_start(out=s_sb[:, :], in_=s_ext.ap()).then_inc(dma_s, 16)
            for t in range(NT):
                if t >= BUFS:
                    # slot reuse: wait until tile t-BUFS fully reduced
                    sync.wait_ge(red_sem, t - BUFS + 1)
                slot = t % BUFS
                sync.dma_start(
                    out=x_buf[:, slot * F:(slot + 1) * F], in_=x_t[t]
                ).then_inc(dma_x[t], 16)
            sync.wait_ge(fin_sem, 1)
            sync.dma_start(out=out_ext.ap(), in_=out_sb[:, :]).then_inc(dma_o, 16)
            sync.wait_ge(dma_o, 16)

        @block.vector
        def _(vector):
            for t in range(NT):
                vector.wait_ge(dma_x[t], 16)
                if t >= XBUFS:
                    # xs slot reuse: wait until tile t-XBUFS consumed by PE
                    vector.wait_ge(pe_sem, t - XBUFS + 1)
                slot = t % BUFS
                xslot = t % XBUFS
                vector.tensor_reduce(
                    out=xs_buf[:, xslot * ML:(xslot + 1) * ML],
                    in_=x_buf[:, slot * F:(slot + 1) * F].rearrange(
                        "p (m j) -> p m j", j=HW
                    ),
                    axis=mybir.AxisListType.X,
                    op=mybir.AluOpType.add,
                ).then_inc(red_sem, 1)
            # epilogue: scale group sums by 1/(count*49)
            vector.wait_ge(dma_s, 16)
            vector.wait_ge(pe_sem, NT)
            vector.tensor_scalar_mul(
                out_sb[:, :], psum[:, :], s_sb[:, 0:1]
            ).then_inc(fin_sem, 1)

        @block.tensor
        def _(tensor):
            tensor.wait_ge(dma_w, 16)
            for t in range(NT):
                tensor.wait_ge(red_sem, t + 1)
                xslot = t % XBUFS
                tensor.matmul(
                    out=psum[:, :],
                    lhsT=w_sb[:, t * G:(t + 1) * G],
                    rhs=xs_buf[:, xslot * ML:(xslot + 1) * ML],
                    start=(t == 0),
                    stop=(t == NT - 1),
                ).then_inc(pe_sem, 1)

    return nc


def _prepare(x, idx):
    x = np.asarray(x)
    if x.dtype != np.float32:
        x = x.astype(np.float32)
    idx = np.asarray(idx).astype(np.int64)
    counts = np.bincount(idx, minlength=G).astype(np.float64)
    scale = np.where(counts > 0, 1.0 / (counts * HW), 0.0).astype(np.float32)
    s_host = np.ascontiguousarray(scale.reshape(G, 1))
    # exact 0/1 one-hot; the per-group scale is applied in the epilogue
    w_full = np.zeros((N, G), np.float32)
    w_full[np.arange(N), idx] = 1.0
    # device layout: w[p, t*G + g] = w_full[t*128 + p, g]
    w_host = np.ascontiguousarray(
        w_full.reshape(NT, P, G).transpose(1, 0, 2).reshape(P, NT * G)
    )
    xr = x.reshape(N, M, HW)
    in_maps = []
    for k in range(CORES):
        shard = np.ascontiguousarray(xr[:, k * ML:(k + 1) * ML, :]).reshape(N, F)
        in_maps.append({"x": shard, "w": w_host, "s": s_host})
    return in_maps


def run(x, tensor_list_assignmentindices, trace=False):
    in_maps = _prepare(x, tensor_list_assignmentindices)
    nc = _build()
    res = run_bass_kernel_spmd(nc, in_maps, core_ids=list(range(CORES)), trace=trace)
    outs = [np.asarray(r["out"]) for r in res.results]
    out = np.concatenate(outs, axis=1)  # [G, M]
    return out.reshape(G, M, 1, 1).astype(np.float32), res.exec_time_ns


def kernel(**inputs):
    out, _ = run(inputs["x"], inputs["tensor_list_assignmentindices"], trace=False)
    return out
